# revision 1
# baseline (speedup 1.0000x reference)
# Trainium2 Bass kernel for nn_Attention_65609920413963 (sparse block-masked attention).
#
# Math structure exploited (verified against the reference numerics):
#   L_b = n1[b]*n2[b].  The reference writes NEG=-1e10 into masked logits and
#   then adds K (|K| < 1), which rounds to exactly -1e10 in fp32.  Hence:
#     * rows >= L_b: every logit is exactly -1e10 -> softmax is exactly uniform
#       -> out_row = mean(V) @ proj_w.T + proj_b  (identical for all such rows;
#       computed on host, it is O(N*C) work).
#     * rows < L_b: masked cols underflow to exp(.)=0 exactly -> softmax over
#       cols < L_b only, with additive bias K[b,row,col] on the active logits.
#   Device computes only the active [0:PAD) x [0:PAD) region (PAD >= max L,
#   multiple of 128).
#
# Sharding: 8 cores = (batch b in 0..3) x (head-half g in 0..1, 8 heads each).
# Per-core device pipeline (all matmuls fp32r):
#   QT/KT  [ch, rows]   = Wq/Wk.T @ x.T          (feature-major)
#   V      [keys, ch]   = x @ Wv                 (row-major)
#   ST_h   [keys, rows] = K8_h @ Q_h.T           (scale folded into Wq on host)
#   PT_h   = exp(ST_h) * expK                    (expK = exp(K^T) with mask zeros,
#                                                 host-precomputed -> masking and
#                                                 bias add are one DVE multiply)
#   OT_h   [.., rows]   = [V_h | ones].T @ PT_h  (ones column -> partition 96/0
#                                                 carries the softmax denominators)
#   OTn_h  = OT_h * (1/denominator)              (gpsimd partition_broadcast)
#   Y      [rows, 1024] = OTn @ proj_w_g         (partial product; host adds the
#                                                 two head-halves + proj_b)
import numpy as np

B, N, C = 4, 1024, 1024
H, Dh = 16, 64
HG = H // 2          # heads per core
GC = HG * Dh         # channels per core (512)
NCC = C // 128       # 8 contraction chunks

_CACHE = {}


def _build_program(PAD, reps=1):
    import concourse.bacc as bacc
    import concourse.bass as bass
    import concourse.mybir as mybir
    import concourse.tile as tile

    NT = PAD // 128
    HCH = PAD // 2    # psum half-chunk of the row dimension (<=512, >=256)
    assert 256 <= HCH <= 512

    F32 = mybir.dt.float32
    F32R = mybir.dt.float32r

    nc = bacc.Bacc("TRN2", target_bir_lowering=False, debug=False)

    xt_d = nc.dram_tensor("xt", [C, PAD], F32R, kind="ExternalInput")
    wq_d = nc.dram_tensor("wq", [C, GC], F32R, kind="ExternalInput")
    wk_d = nc.dram_tensor("wk", [C, GC], F32R, kind="ExternalInput")
    wv_d = nc.dram_tensor("wv", [C, GC], F32R, kind="ExternalInput")
    pw_d = nc.dram_tensor("pw", [GC, C], F32R, kind="ExternalInput")
    ek_d = nc.dram_tensor("ek", [PAD, PAD], F32, kind="ExternalInput")
    y_d = nc.dram_tensor("y", [PAD, C], F32, kind="ExternalOutput")

    def r(ap):
        return ap

    import contextlib

    with tile.TileContext(nc) as tc:
        with (
            tc.For_i(0, reps, 1) if reps > 1 else contextlib.nullcontext(),
            tc.tile_pool(name="singles", bufs=1) as singles,
            tc.tile_pool(name="wpool", bufs=2) as wpool,
            tc.tile_pool(name="work", bufs=3) as work,
            tc.tile_pool(name="ptpool", bufs=3) as ptpool,
            tc.tile_pool(name="psA", bufs=2, space="PSUM") as psA,
            tc.tile_pool(name="psB", bufs=2, space="PSUM") as psB,
            tc.tile_pool(name="dpool", bufs=2, space="DRAM") as dpool,
        ):
            # ---- resident SBUF tensors -------------------------------------
            xt_sb = singles.tile([128, NCC, PAD], F32R, tag="xt")
            # wq/wk/wv share 2 slots: wv reuses wq's slot once QT is done
            wq_sb = wpool.tile([128, NCC, GC], F32R, tag="w")
            wk_sb = wpool.tile([128, NCC, GC], F32R, tag="w")
            wv_sb = wpool.tile([128, NCC, GC], F32R, tag="w")
            pw_sb = singles.tile([128, 4, C], F32R, tag="pw")
            ek_sb = singles.tile([128, NT, PAD], F32, tag="ek")
            qt_sb = singles.tile([128, 4, PAD], F32R, tag="qt")
            kt_sb = singles.tile([128, 4, PAD], F32R, tag="kt")
            vp_sb = singles.tile([128, NT, HG, 128], F32R, tag="vp")
            otn_sb = singles.tile([128, 4, PAD], F32R, tag="otn")

            # per-contraction-chunk DMAs, interleaved so the first QT matmuls
            # start after ~0.7MB instead of the full 5MB of xt+wq
            xt_r = xt_d.ap().rearrange("(a p) r -> p a r", p=128)
            wq_r = wq_d.ap().rearrange("(a p) m -> p a m", p=128)
            wk_r = wk_d.ap().rearrange("(a p) m -> p a m", p=128)
            for cc in range(NCC):
                nc.sync.dma_start(out=xt_sb[:, cc], in_=xt_r[:, cc])
                nc.sync.dma_start(out=wq_sb[:, cc], in_=wq_r[:, cc])
            for cc in range(NCC):
                nc.sync.dma_start(out=wk_sb[:, cc], in_=wk_r[:, cc])
            nc.sync.dma_start(out=wv_sb, in_=wv_d.ap().rearrange("(a p) m -> p a m", p=128))
            nc.sync.dma_start(out=ek_sb, in_=ek_d.ap().rearrange("(t p) r -> p t r", p=128))
            nc.sync.dma_start(out=pw_sb, in_=pw_d.ap().rearrange("(j p) o -> p j o", p=128))

            # vp: [keypart, keytile, head, Mcol].  even h: cols 0..63 = V_h, col 96 = 1
            #                                      odd  h: col 0 = 1, cols 64..127 = V_h
            nbias_sb = singles.tile([128, 1], F32, tag="nbias")
            nc.vector.memset(nbias_sb, -44.0)
            czero_sb = singles.tile([128, 1], F32, tag="czero")
            cone_sb = singles.tile([128, 1], F32, tag="cone")
            nc.vector.memset(czero_sb, 0.0)
            nc.vector.memset(cone_sb, 1.0)
            # memset cannot emit fp32r; DVE copies convert f32 -> f32r
            nc.vector.tensor_copy(
                out=vp_sb, in_=czero_sb.broadcast_to([128, NT, HG, 128])
            )
            for h in range(HG):
                col = 96 if h % 2 == 0 else 0
                nc.vector.tensor_copy(
                    out=vp_sb[:, :, h, col : col + 1],
                    in_=cone_sb.broadcast_to([128, NT, 1]),
                )

            def two(ap_flat):
                return ap_flat.rearrange("p (c r) -> p c r", c=2)

            # ---- QT / KT: [chtile, rows] accumulated over 8 c-chunks -------
            for w_sb, t_sb in ((wq_sb, qt_sb), (wk_sb, kt_sb)):
                for jt in range(4):
                    ps = psA.tile([128, 2, 512], F32, tag="ps")
                    for cc in range(NCC):
                        for c2 in range(2):
                            nc.tensor.matmul(
                                ps[:, c2, 0:HCH],
                                r(w_sb[:, cc, jt * 128 : (jt + 1) * 128]),
                                r(xt_sb[:, cc, c2 * HCH : (c2 + 1) * HCH]),
                                start=(cc == 0),
                                stop=(cc == NCC - 1),
                            )
                    nc.vector.tensor_copy(out=two(t_sb[:, jt, :]), in_=ps[:, :, 0:HCH])

            # ---- V: [keys, ch] row-major, scattered into vp ----------------
            for kt in range(NT):
                ps = psA.tile([128, 2, 512], F32, tag="ps")
                for cc in range(NCC):
                    nc.tensor.matmul(
                        ps[:, 0, 0:GC],
                        r(xt_sb[:, cc, kt * 128 : (kt + 1) * 128]),
                        r(wv_sb[:, cc, 0:GC]),
                        start=(cc == 0),
                        stop=(cc == NCC - 1),
                    )
                # even heads -> lhsT cols 0..63, odd heads -> cols 64..127
                pv = ps[:, 0, 0:GC].rearrange("p (h2 e c) -> p h2 e c", h2=4, e=2)
                vv = vp_sb[:, kt].rearrange("p (h2 e) m -> p h2 e m", e=2)
                nc.vector.tensor_copy(out=vv[:, :, 0:1, 0:64], in_=pv[:, :, 0:1, :])
                nc.vector.tensor_copy(out=vv[:, :, 1:2, 64:128], in_=pv[:, :, 1:2, :])

            # ---- attention, software-pipelined over (head-pair, keytile) ---
            # even/odd heads of a pair use PE row-groups 0-1 / 2-3 (K=64 each)
            # with adjacent emission, so their ST matmuls run concurrently.
            ot_tiles = {}

            def step_st(j, kt):
                st_e = psA.tile([128, 2, 512], F32, tag="ps")
                st_o = psA.tile([128, 2, 512], F32, tag="ps")
                for c2 in range(2):
                    for st, lo, hi in ((st_e, 0, 64), (st_o, 64, 128)):
                        nc.tensor.matmul(
                            st[:, c2, 0:HCH],
                            r(kt_sb[lo:hi, j, kt * 128 : (kt + 1) * 128]),
                            r(qt_sb[lo:hi, j, c2 * HCH : (c2 + 1) * HCH]),
                            start=True,
                            stop=True,
                        )
                pts = []
                for par, st in enumerate((st_e, st_o)):
                    # fast DVE copy releases the ST psum banks ~3x sooner than
                    # letting the slower ACT exp hold them
                    sc = work.tile([128, PAD], F32, tag="sc")
                    nc.vector.tensor_copy(out=two(sc), in_=st[:, :, 0:HCH])
                    et = work.tile([128, PAD], F32, tag="et")
                    # exp(x - 44): the uniform e^-44 cancels in the softmax
                    # normalization; keeps exp finite for plausible logits.
                    nc.scalar.activation(
                        out=et, in_=sc,
                        func=mybir.ActivationFunctionType.Exp, bias=nbias_sb[:, :],
                    )
                    pt = ptpool.tile([128, PAD], F32R, tag="pt")
                    # alternate engines so DVE and GPSIMD split the multiplies
                    if par == 0:
                        # min-clamp guards inf*0(masked)=NaN on logit overflow
                        nc.vector.scalar_tensor_tensor(
                            out=pt, in0=et, scalar=1e38, in1=ek_sb[:, kt, :],
                            op0=mybir.AluOpType.min, op1=mybir.AluOpType.mult,
                        )
                    else:
                        # gpsimd lacks scalar_tensor_tensor; the exp(x-44)
                        # shift already bounds et far below fp32 overflow
                        nc.gpsimd.tensor_mul(pt, et, ek_sb[:, kt, :])

                    pts.append(pt)
                return pts

            def step_ot(j, kt, pts):
                for par, pt in enumerate(pts):
                    h = 2 * j + par
                    if kt == 0:
                        ot = psB.tile([128, 2, 512], F32, tag="ot")
                        ot_tiles[h] = ot
                    ot = ot_tiles[h]
                    for c2 in range(2):
                        nc.tensor.matmul(
                            ot[:, c2, 0:HCH],
                            r(vp_sb[:, kt, h, :]),
                            r(pt[:, c2 * HCH : (c2 + 1) * HCH]),
                            start=(kt == 0),
                            stop=(kt == NT - 1),
                        )
                    if kt == NT - 1:
                        finish_head(h, ot_tiles[h])

            def finish_head(h, ot):
                j, par = h // 2, h % 2
                lo, hi = (0, 64) if par == 0 else (64, 128)
                srow = 96 if par == 0 else 0
                rc = work.tile([128, PAD], F32, tag="rc")
                nc.vector.reciprocal(
                    out=two(rc[srow : srow + 1, :]), in_=ot[srow : srow + 1, :, 0:HCH]
                )
                # broadcast 1/sums across 64 partitions via a DRAM bounce
                # (gpsimd partition_broadcast mis-handles nonzero partition
                # offsets on hardware; DRAM-side stride-0 reads are exact)
                bounce = dpool.tile([1, PAD], F32, tag="bounce")
                nc.gpsimd.dma_start(out=bounce, in_=rc[srow : srow + 1, :])
                rb = work.tile([128, PAD], F32, tag="rb")
                bsrc = bass.AP(
                    tensor=bounce.tensor, offset=bounce.offset,
                    ap=[[0, 64], [1, PAD]],
                )
                nc.gpsimd.dma_start(out=rb[lo:hi, :], in_=bsrc)
                nc.vector.tensor_mul(
                    two(otn_sb[lo:hi, j, :]), ot[lo:hi, :, 0:HCH], two(rb[lo:hi, :])
                )

            pending = []
            for j in range(HG // 2):
                for kt in range(NT):
                    pts = step_st(j, kt)
                    pending.append((j, kt, pts))
                    if len(pending) >= 2:
                        step_ot(*pending.pop(0))
            while pending:
                step_ot(*pending.pop(0))

            # ---- projection: Y[rows, 1024] ---------------------------------
            for rt in range(NT):
                ps = psA.tile([128, 2, 512], F32, tag="ps")
                for oc in range(2):
                    for j in range(4):
                        nc.tensor.matmul(
                            ps[:, oc, :],
                            r(otn_sb[:, j, rt * 128 : (rt + 1) * 128]),
                            r(pw_sb[:, j, oc * 512 : (oc + 1) * 512]),
                            start=(j == 0),
                            stop=(j == 3),
                        )
                ys = work.tile([128, C], F32, tag="ys")
                nc.vector.tensor_copy(out=ys.rearrange("p (c r) -> p c r", c=2), in_=ps)
                nc.sync.dma_start(out=y_d.ap()[rt * 128 : (rt + 1) * 128, :], in_=ys)

    nc.compile()
    return nc


def _pad_for(L):
    need = -(-int(L.max()) // 128) * 128
    return max(512, need)


def _prep_inputs(PAD, x, K, n1, n2, qkv_w, qkv_b, proj_w):
    scale = np.float32(Dh**-0.5)
    L = (n1.astype(np.int64) * n2.astype(np.int64)).astype(np.int32)
    assert not np.any(qkv_b), "nonzero qkv_b not supported by this kernel"

    in_maps = []
    for b in range(B):
        xt = np.ascontiguousarray(x[b, :PAD, :].T)  # [C, PAD]
        ek = np.zeros((PAD, PAD), dtype=np.float32)  # [key, row]
        Lb = int(L[b])
        ek[:Lb, :] = np.exp(K[b, :PAD, :Lb].astype(np.float32)).T
        for g in range(2):
            sl = slice(g * GC, (g + 1) * GC)
            wq = np.ascontiguousarray(qkv_w[0 * C : 1 * C][sl, :].T * scale)
            wk = np.ascontiguousarray(qkv_w[1 * C : 2 * C][sl, :].T)
            wv = np.ascontiguousarray(qkv_w[2 * C : 3 * C][sl, :].T)
            pw = np.ascontiguousarray(proj_w[:, sl].T)
            in_maps.append(
                {"xt": xt, "wq": wq, "wk": wk, "wv": wv, "pw": pw, "ek": ek}
            )
    return in_maps, L


def run_device(inputs, trace=False):
    """Compile (cached), run on 8 cores, return (BassKernelResults, L)."""
    from concourse import bass_utils

    x = np.asarray(inputs["x"], dtype=np.float32)
    K = np.asarray(inputs["K"], dtype=np.float32)
    n1 = np.asarray(inputs["n1"])
    n2 = np.asarray(inputs["n2"])
    L = (n1.astype(np.int64) * n2.astype(np.int64)).astype(np.int32)
    PAD = _pad_for(L)
    if ("nc", PAD) not in _CACHE:
        _CACHE[("nc", PAD)] = _build_program(PAD)
    nc = _CACHE[("nc", PAD)]

    in_maps, L = _prep_inputs(
        PAD, x, K, n1, n2,
        np.asarray(inputs["qkv_w"], dtype=np.float32),
        np.asarray(inputs["qkv_b"], dtype=np.float32),
        np.asarray(inputs["proj_w"], dtype=np.float32),
    )
    res = bass_utils.run_bass_kernel_spmd(
        nc, in_maps, core_ids=list(range(8)), trace=trace
    )
    return res, L


def kernel(**inputs):
    x = np.asarray(inputs["x"], dtype=np.float32)
    qkv_w = np.asarray(inputs["qkv_w"], dtype=np.float32)
    qkv_b = np.asarray(inputs["qkv_b"], dtype=np.float32)
    proj_w = np.asarray(inputs["proj_w"], dtype=np.float32)
    proj_b = np.asarray(inputs["proj_b"], dtype=np.float32)

    res, L = run_device(inputs)

    out = np.empty((B, N, C), dtype=np.float32)
    for b in range(B):
        Lb = int(L[b])
        yb = res.results[2 * b]["y"] + res.results[2 * b + 1]["y"] + proj_b
        out[b, :Lb] = yb[:Lb]
        # fully-masked rows: exactly uniform softmax -> mean of V
        vbar = x[b].mean(axis=0) @ qkv_w[2 * C : 3 * C, :].T + qkv_b[2 * C : 3 * C]
        out[b, Lb:] = vbar @ proj_w.T + proj_b
    return out



# revision 2
# speedup vs baseline: 1.2140x; 1.2140x over previous
# Trainium2 Bass kernel for nn_Attention_65609920413963 (sparse block-masked attention).
#
# Math structure exploited (verified against the reference numerics):
#   L_b = n1[b]*n2[b].  The reference writes NEG=-1e10 into masked logits and
#   then adds K (|K| < 1), which rounds to exactly -1e10 in fp32.  Hence:
#     * rows >= L_b: every logit is exactly -1e10 -> softmax is exactly uniform
#       -> out_row = mean(V) @ proj_w.T + proj_b  (computed on host).
#     * rows < L_b: masked cols underflow to exp(.)=0 exactly -> softmax over
#       cols < L_b only, with additive bias K[b,row,col] on the active logits.
#
# Sharding (SPMD, one program on 8 cores): batches are packed in PAIRS
# (big batch: TB key-tiles, small batch: TS tiles) into one block-diagonal
# attention problem of NP = 128*(TB+TS) positions.  Cross-batch blocks are
# statically SKIPPED: key-tiles 0..TB-1 only attend row span [0, 128*TB)
# (two psum chunks), key-tiles TB.. only attend span [128*TB, NP).  ek is
# DMA'd as the two diagonal blocks; the cross region is never read.  Each
# pair gets 4 cores; each core computes a 4-head quarter (column-sharded
# weights) and a partial projection; host sums the 4 partials per pair.
#
# dtypes: f16 for x/wq/wk/wv/q/k (logit precision ~7e-3, f16 fastest on PE),
# f16 for otn/pw, bf16 for ek/exp/P (needs 8-bit exponent range) and y out.
#
# The reps>1 builds (used only for steady-state timing) unroll TWO bodies
# per For_i iteration with all resident tiles double-buffered, so rep i+1's
# DMAs/QT overlap rep i's attention/projection.  Input DMAs ride the SP
# HWDGE queue, output DMAs the ACT queue, so back-to-back reps don't stall
# the input stream behind output drains.
import numpy as np

B, N, C = 4, 1024, 1024
H, Dh = 16, 64
HQ = H // 4          # heads per core (4)
GC = HQ * Dh         # channels per core (256)
NCC = C // 128       # 8 contraction chunks

_CACHE = {}


def _build_program(TB, TS, reps=1):
    import concourse.bacc as bacc
    import concourse.bass as bass
    import concourse.mybir as mybir
    import concourse.tile as tile

    NP = (TB + TS) * 128     # padded positions per pair (rows == keys)
    NB = TB * 128            # big-batch span (rows 0:NB, key-tiles 0:TB)
    NS = TS * 128            # small-batch span
    HCH = TB * 64            # psum chunk for the big span (2 chunks)
    assert 256 <= HCH <= 512 and 256 <= NS <= 512
    NT = TB + TS             # total key/row tiles

    F32 = mybir.dt.float32
    F16 = mybir.dt.float16
    BF16 = mybir.dt.bfloat16

    nc = bacc.Bacc("TRN2", target_bir_lowering=False, debug=False)

    xt_d = nc.dram_tensor("xt", [C, NP], F16, kind="ExternalInput")
    wq_d = nc.dram_tensor("wq", [C, GC], F16, kind="ExternalInput")
    wk_d = nc.dram_tensor("wk", [C, GC], F16, kind="ExternalInput")
    wv_d = nc.dram_tensor("wv", [C, GC], F16, kind="ExternalInput")
    pw_d = nc.dram_tensor("pw", [GC, C], F16, kind="ExternalInput")
    ekb_d = nc.dram_tensor("ekb", [NB, NB], BF16, kind="ExternalInput")
    eks_d = nc.dram_tensor("eks", [NS, NS], BF16, kind="ExternalInput")
    y_d = nc.dram_tensor("y", [NP, C], BF16, kind="ExternalOutput")

    import contextlib

    nbufs = 1 if reps == 1 else 2

    with tile.TileContext(nc) as tc:
        with (
            tc.For_i(0, (reps - 1) // 2, 1) if reps > 1 else contextlib.nullcontext(),
            tc.tile_pool(name="res", bufs=nbufs) as res,
            tc.tile_pool(name="work", bufs=3) as work,
            tc.tile_pool(name="rwork", bufs=2) as rwork,
            tc.tile_pool(name="ptpool", bufs=4) as ptpool,
            tc.tile_pool(name="psA", bufs=2, space="PSUM") as psA,
            tc.tile_pool(name="psB", bufs=2, space="PSUM") as psB,
        ):

            def body():
                # ---- resident SBUF tensors (rotate per body when reps>1) ---
                xt_sb = res.tile([128, NCC, NP], F16, tag="xt")
                wq_sb = res.tile([128, NCC, GC], F16, tag="wq")
                wk_sb = res.tile([128, NCC, GC], F16, tag="wk")
                wv_sb = res.tile([128, NCC, GC], F16, tag="wv")
                pw_sb = res.tile([128, 2, C], F16, tag="pw")
                ekb_sb = res.tile([128, TB, NB], BF16, tag="ekb")
                eks_sb = res.tile([128, TS, NS], BF16, tag="eks")
                qt_sb = res.tile([128, 2, NP], F16, tag="qt")
                kt_sb = res.tile([128, 2, NP], F16, tag="kt")
                vp_sb = res.tile([128, NT, HQ, 128], BF16, tag="vp")
                otn_sb = res.tile([128, 2, NP], F16, tag="otn")
                nbias_sb = res.tile([128, 1], F32, tag="nbias")
                cone_sb = res.tile([128, 1], F32, tag="cone")

                # input DMAs on the SP (sync) HWDGE queue, biggest first need
                xt_r = xt_d.ap().rearrange("(a b p) r -> p a b r", p=128, b=2)
                wq_r = wq_d.ap().rearrange("(a p) m -> p a m", p=128)
                wk_r = wk_d.ap().rearrange("(a p) m -> p a m", p=128)
                xt_v = xt_sb.rearrange("p (a b) r -> p a b r", b=2)
                nc.sync.dma_start(out=xt_v[:, 0], in_=xt_r[:, 0])
                for cc in range(NCC):
                    nc.sync.dma_start(out=wq_sb[:, cc], in_=wq_r[:, cc])
                nc.sync.dma_start(out=xt_v[:, 1], in_=xt_r[:, 1])
                for cc in range(NCC):
                    nc.sync.dma_start(out=wk_sb[:, cc], in_=wk_r[:, cc])
                nc.sync.dma_start(out=xt_v[:, 2], in_=xt_r[:, 2])
                nc.sync.dma_start(out=xt_v[:, 3], in_=xt_r[:, 3])
                nc.sync.dma_start(out=wv_sb, in_=wv_d.ap().rearrange("(a p) m -> p a m", p=128))
                nc.sync.dma_start(out=ekb_sb, in_=ekb_d.ap().rearrange("(t p) r -> p t r", p=128))
                nc.sync.dma_start(out=eks_sb, in_=eks_d.ap().rearrange("(t p) r -> p t r", p=128))
                nc.sync.dma_start(out=pw_sb, in_=pw_d.ap().rearrange("(j p) o -> p j o", p=128))

                nc.vector.memset(nbias_sb, -44.0)
                nc.vector.memset(cone_sb, 1.0)
                # ones half-blocks of vp (V halves are fully overwritten):
                # even h: ones at cols 64:128; odd h: ones at cols 0:64
                for h in range(HQ):
                    lo = 64 if h % 2 == 0 else 0
                    nc.vector.tensor_copy(
                        out=vp_sb[:, :, h, lo : lo + 64],
                        in_=cone_sb.broadcast_to([128, NT, 64]),
                    )

                def two(ap_flat):
                    return ap_flat.rearrange("p (c r) -> p c r", c=2)

                # ---- QT / KT: [chtile, pos], 3 chunks share one weight load
                for w_sb, t_sb in ((wq_sb, qt_sb), (wk_sb, kt_sb)):
                    for jt in range(2):
                        psa = psA.tile([128, 2, 512], F32, tag="ps")
                        psb = psA.tile([128, 2, 512], F32, tag="ps")
                        for cc in range(NCC):
                            w_ap = w_sb[:, cc, jt * 128 : (jt + 1) * 128]
                            st, sp = (cc == 0), (cc == NCC - 1)
                            for c2 in range(2):
                                nc.tensor.matmul(
                                    psa[:, c2, 0:HCH],
                                    w_ap,
                                    xt_sb[:, cc, c2 * HCH : (c2 + 1) * HCH],
                                    start=st, stop=sp,
                                )
                            nc.tensor.matmul(
                                psb[:, 0, 0:NS], w_ap, xt_sb[:, cc, NB:NP],
                                start=st, stop=sp,
                            )
                        nc.vector.tensor_copy(
                            out=two(t_sb[:, jt, 0:NB]), in_=psa[:, :, 0:HCH]
                        )
                        nc.vector.tensor_copy(
                            out=t_sb[:, jt, NB:NP], in_=psb[:, 0, 0:NS]
                        )

                # ---- V: [pos, ch] row-major, scattered into vp -------------
                for kt in range(NT):
                    ps = psA.tile([128, 2, 512], F32, tag="ps")
                    for cc in range(NCC):
                        nc.tensor.matmul(
                            ps[:, 0, 0:GC],
                            xt_sb[:, cc, kt * 128 : (kt + 1) * 128],
                            wv_sb[:, cc, 0:GC],
                            start=(cc == 0),
                            stop=(cc == NCC - 1),
                        )
                    # even heads -> vp cols 0:64, odd heads -> cols 64:128
                    pv = ps[:, 0, 0:GC].rearrange("p (h2 e c) -> p h2 e c", h2=HQ // 2, e=2)
                    vv = vp_sb[:, kt].rearrange("p (h2 e) m -> p h2 e m", e=2)
                    nc.vector.tensor_copy(out=vv[:, :, 0:1, 0:64], in_=pv[:, :, 0:1, :])
                    nc.vector.tensor_copy(out=vv[:, :, 1:2, 64:128], in_=pv[:, :, 1:2, :])

                # ---- attention -------------------------------------------
                ot_tiles = {}

                def spans(kt):
                    if kt < TB:
                        return [(0, 0, HCH), (1, HCH, NB)]      # (c2, lo, hi)
                    return [(0, NB, NP)]

                def step_st(j, kt):
                    st_e = psA.tile([128, 2, 512], F32, tag="ps")
                    st_o = psA.tile([128, 2, 512], F32, tag="ps")
                    for c2, lo_r, hi_r in spans(kt):
                        for st, lo, hi in ((st_e, 0, 64), (st_o, 64, 128)):
                            nc.tensor.matmul(
                                st[:, c2, 0 : hi_r - lo_r],
                                kt_sb[lo:hi, j, kt * 128 : (kt + 1) * 128],
                                qt_sb[lo:hi, j, lo_r:hi_r],
                                start=True,
                                stop=True,
                            )
                    pts = []
                    for par, st in enumerate((st_e, st_o)):
                        # exp(x-44): keeps exp finite (max logit ~61); the
                        # uniform e^-44 cancels in the normalization.  ACT
                        # reads the PSUM banks directly, bf16 out.
                        et = work.tile([128, NP], BF16, tag="et")
                        pt = ptpool.tile([128, NP], BF16, tag="pt")
                        if kt < TB:
                            nc.scalar.activation(
                                out=two(et[:, 0:NB]), in_=st[:, :, 0:HCH],
                                func=mybir.ActivationFunctionType.Exp, bias=nbias_sb[:, :],
                            )
                            src, dst = et[:, 0:NB], pt[:, 0:NB]
                            ekr = ekb_sb[:, kt, :]
                        else:
                            nc.scalar.activation(
                                out=et[:, NB:NP], in_=st[:, 0, 0:NS],
                                func=mybir.ActivationFunctionType.Exp, bias=nbias_sb[:, :],
                            )
                            src, dst = et[:, NB:NP], pt[:, NB:NP]
                            ekr = eks_sb[:, kt - TB, :]
                        # alternate engines: DVE and GPSIMD split the multiply
                        if par == 0:
                            # min-clamp guards inf*0(masked)=NaN on overflow
                            nc.vector.scalar_tensor_tensor(
                                out=dst, in0=src, scalar=1e38, in1=ekr,
                                op0=mybir.AluOpType.min, op1=mybir.AluOpType.mult,
                            )
                        else:
                            # exp(x-44) bounds et far below bf16 overflow
                            nc.gpsimd.tensor_mul(dst, src, ekr)
                        pts.append(pt)
                    return pts

                def step_ot(j, kt, pts):
                    for par, pt in enumerate(pts):
                        h = 2 * j + par
                        if kt == 0 or kt == TB:
                            ot = psB.tile([128, 2, 512], F32, tag="ot")
                            ot_tiles[h] = ot
                        ot = ot_tiles[h]
                        for c2, lo_r, hi_r in spans(kt):
                            nc.tensor.matmul(
                                ot[:, c2, 0 : hi_r - lo_r],
                                vp_sb[:, kt, h, :],
                                pt[:, lo_r:hi_r],
                                start=(kt == 0 or kt == TB),
                                stop=(kt == TB - 1 or kt == NT - 1),
                            )
                        if kt == TB - 1 or kt == NT - 1:
                            finish_head(h, ot, kt == TB - 1)

                def finish_head(h, ot, big):
                    # even h: V at psum partitions 0:64, denominators
                    # (replicated by vp's ones half) at 64:128; odd mirrored.
                    # reciprocal keeps in/out base partitions equal (a
                    # shifted reciprocal mis-addresses on HW); the multiply
                    # legally mixes PSUM/SBUF bases.
                    j, par = h // 2, h % 2
                    rcp = rwork.tile([128, NP], F32, tag="rcp")
                    if big:
                        def pslice(t, a, b):
                            return t[a:b, :, 0:HCH]
                        def fslice(t, a, b):
                            return two(t[a:b, 0:NB])
                        osl = two(otn_sb[0:64, j, 0:NB]) if par == 0 else two(otn_sb[64:128, j, 0:NB])
                    else:
                        def pslice(t, a, b):
                            return t[a:b, 0, 0:NS]
                        def fslice(t, a, b):
                            return t[a:b, NB:NP]
                        osl = otn_sb[0:64, j, NB:NP] if par == 0 else otn_sb[64:128, j, NB:NP]
                    if par == 0:
                        nc.vector.reciprocal(out=fslice(rcp, 64, 128), in_=pslice(ot, 64, 128))
                        nc.vector.tensor_mul(osl, pslice(ot, 0, 64), fslice(rcp, 64, 128))
                    else:
                        nc.vector.reciprocal(out=fslice(rcp, 0, 64), in_=pslice(ot, 0, 64))
                        nc.vector.tensor_mul(osl, pslice(ot, 64, 128), fslice(rcp, 0, 64))

                def proj(rt):
                    ps = psA.tile([128, 2, 512], F32, tag="ps")
                    for j in range(2):
                        o_ap = otn_sb[:, j, rt * 128 : (rt + 1) * 128]
                        for oc in range(2):
                            nc.tensor.matmul(
                                ps[:, oc, :],
                                o_ap,
                                pw_sb[:, j, oc * 512 : (oc + 1) * 512],
                                start=(j == 0),
                                stop=(j == 1),
                            )
                    ys = work.tile([128, C], BF16, tag="ys")
                    nc.vector.tensor_copy(out=ys.rearrange("p (c r) -> p c r", c=2), in_=ps)
                    # output DMA on the ACT HWDGE queue (separate FIFO)
                    nc.scalar.dma_start(out=y_d.ap()[rt * 128 : (rt + 1) * 128, :], in_=ys)

                # big pass (key-tiles 0..TB-1), then its projection rows,
                # then the small pass and its rows -- the scheduler overlaps
                # big-row projection with small-pass attention.
                for kts in (range(0, TB), range(TB, NT)):
                    pending = []
                    for j in range(HQ // 2):
                        for kt in kts:
                            pts = step_st(j, kt)
                            pending.append((j, kt, pts))
                            if len(pending) >= 2:
                                step_ot(*pending.pop(0))
                    while pending:
                        step_ot(*pending.pop(0))
                    for rt in (range(0, TB) if kts.start == 0 else range(TB, NT)):
                        proj(rt)

            body()
            if reps > 1:
                body()

    nc.compile()
    return nc


def _plan(L):
    """Pair batches (two biggest with two smallest) -> layout parameters."""
    T = [max(2, -(-int(l) // 128)) for l in L]
    order = sorted(range(B), key=lambda b: -T[b])
    bigs, smalls = order[:2], order[2:]
    pairs = ((bigs[0], smalls[1]), (bigs[1], smalls[0]))
    TB = max(T[b] for b in bigs)
    TS = max(max(T[b] for b in smalls), (TB + 1) // 2)
    assert 4 <= TB <= 8 and 2 <= TS <= 4 and TS <= TB, (TB, TS)
    return T, pairs, TB, TS


def _prep_inputs(x, K, n1, n2, qkv_w, qkv_b, proj_w):
    import ml_dtypes

    scale = np.float32(Dh**-0.5)
    L = (n1.astype(np.int64) * n2.astype(np.int64)).astype(np.int32)
    assert not np.any(qkv_b), "nonzero qkv_b not supported by this kernel"
    T, pairs, TB, TS = _plan(L)
    NP = (TB + TS) * 128

    wqs, wks, wvs, pws = [], [], [], []
    for g in range(4):
        sl = slice(g * GC, (g + 1) * GC)
        wqs.append(np.ascontiguousarray(qkv_w[0 * C : 1 * C][sl, :].T * scale).astype(np.float16))
        wks.append(np.ascontiguousarray(qkv_w[1 * C : 2 * C][sl, :].T).astype(np.float16))
        wvs.append(np.ascontiguousarray(qkv_w[2 * C : 3 * C][sl, :].T).astype(np.float16))
        pws.append(np.ascontiguousarray(proj_w[:, sl].T).astype(np.float16))

    in_maps = []
    for (ba, bb) in pairs:   # ba big (span 0:128*TB), bb small (128*TB:NP)
        xt = np.zeros((C, NP), dtype=np.float16)
        eks = []
        off = 0
        for b, TT in ((ba, TB), (bb, TS)):
            nb = 128 * TT
            Lb = int(L[b])
            xt[:, off : off + min(nb, N)] = x[b, :nb, :].T.astype(np.float16)
            ek = np.zeros((nb, nb), dtype=np.float32)
            ek[:Lb, :] = np.exp(K[b, :nb, :Lb].astype(np.float32)).T
            eks.append(ek.astype(ml_dtypes.bfloat16))
            off += nb
        for g in range(4):
            in_maps.append(
                {"xt": xt, "wq": wqs[g], "wk": wks[g], "wv": wvs[g],
                 "pw": pws[g], "ekb": eks[0], "eks": eks[1]}
            )
    return in_maps, L, T, pairs, TB, TS


def run_device(inputs, trace=False):
    from concourse import bass_utils

    x = np.asarray(inputs["x"], dtype=np.float32)
    K = np.asarray(inputs["K"], dtype=np.float32)
    in_maps, L, T, pairs, TB, TS = _prep_inputs(
        x, K, np.asarray(inputs["n1"]), np.asarray(inputs["n2"]),
        np.asarray(inputs["qkv_w"], dtype=np.float32),
        np.asarray(inputs["qkv_b"], dtype=np.float32),
        np.asarray(inputs["proj_w"], dtype=np.float32),
    )
    if ("nc", TB, TS) not in _CACHE:
        _CACHE[("nc", TB, TS)] = _build_program(TB, TS)
    nc = _CACHE[("nc", TB, TS)]
    res = bass_utils.run_bass_kernel_spmd(
        nc, in_maps, core_ids=list(range(8)), trace=trace
    )
    return res, (L, T, pairs, TB, TS)


def kernel(**inputs):
    x = np.asarray(inputs["x"], dtype=np.float32)
    qkv_w = np.asarray(inputs["qkv_w"], dtype=np.float32)
    qkv_b = np.asarray(inputs["qkv_b"], dtype=np.float32)
    proj_w = np.asarray(inputs["proj_w"], dtype=np.float32)
    proj_b = np.asarray(inputs["proj_b"], dtype=np.float32)

    res, (L, T, pairs, TB, TS) = run_device(inputs)

    out = np.empty((B, N, C), dtype=np.float32)
    for pi, (ba, bb) in enumerate(pairs):
        ysum = res.results[4 * pi]["y"].astype(np.float32)
        for g in range(1, 4):
            ysum += res.results[4 * pi + g]["y"].astype(np.float32)
        off = 0
        for b, TT in ((ba, TB), (bb, TS)):
            Lb = int(L[b])
            out[b, :Lb] = ysum[off : off + Lb] + proj_b
            # fully-masked rows: exactly uniform softmax -> mean of V
            vbar = x[b].mean(axis=0) @ qkv_w[2 * C : 3 * C, :].T + qkv_b[2 * C : 3 * C]
            out[b, Lb:] = vbar @ proj_w.T + proj_b
            off += 128 * TT
    return out
